# revision 53
# baseline (speedup 1.0000x reference)
"""AttentionBlock (GroupNorm(32) + 1-head self-attention + proj + residual) on 8 trn2 cores.

Data-parallel over batch: each of the 8 NeuronCores processes 2 of the 16 images.

Algebraic fusion (valid because the reference's q/k biases are zero):
  scores: s_ij = q_i.k_j = xn_i^T (Wq^T Wk) xn_j. Precompute M = Wq^T Wk
          host-side, compute m = M^T xn on device (ONE projection instead of
          q and k), then s_ij = m_i . xn_j with xn as the stationary operand.
  output: proj(attn@V) = sum_j p_j (Wp Wv xn_j). Precompute W' = Wp Wv, so
          u = W' xn replaces v and the separate projection matmul vanishes.

All large matmuls run in fp8e4 with perf_mode=DoubleRow (K=256 per pass).
Scaling scheme: M and W' are scaled by 16 host-side; the softmax scale c^-0.5
and the 1/16 fold into the exp activation's scale; a -2.0 bias keeps exp
outputs <= ~40 (cancels in softmax). u carries 16x, which cancels against the
all-16s matmul used for the softmax denominator (recip of 16*l).

Engine balancing (v2): GroupNorm statistics via DVE bn_stats/bn_aggr (one
pass over x, no PSUM stats matmuls, no x^2 materialization); per-channel
(mean, E[x^2]) reduced to groups by tiny f32r indicator matmuls. PSUM tiles
are [128,1024] (2 banks) so PSUM->SBUF evacuations and exp run as single
1024-wide instructions. Evacuations and the fin tail are split across
ACT/DVE/Pool (gpsimd) per the engine-assignment tables below; Pool was idle
in the baseline. Emission interleaves the two images so ACT (exp+xn, the
busiest engine) stays fed: img b's bn_stats run under img a's attention, and
img a's attn@U runs while ACT converts xn_b.
"""

import ml_dtypes
import numpy as np

import concourse.bacc as bacc
import concourse.tile as tile
import concourse.mybir as mybir
from concourse.bass_utils import run_bass_kernel_spmd

F32 = mybir.dt.float32
F32R = mybir.dt.float32r
F8 = mybir.dt.float8e4
I32 = mybir.dt.int32
AF = mybir.ActivationFunctionType
ALU = mybir.AluOpType
AX = mybir.AxisListType
DR = mybir.MatmulPerfMode.DoubleRow

B, C, H, W = 16, 512, 32, 32
N = H * W                 # 1024 positions
NCORES = 8
BPC = B // NCORES         # 2 images per core
G = 32                    # groupnorm groups
GS = C // G               # 16 channels per group
CT = C // 128             # 4 channel tiles
NT = N // 128             # 8 position tiles
EPS = 1e-5
SCALE = float(C) ** -0.5  # single head, head_dim = C
WSC = 16.0                # host-side weight scale (power of 2, exact in fp8)
EXP_SCALE = SCALE / WSC   # m carries 16x; exp undoes it + softmax scale
EXP_BIAS = -2.0
MAGIC = 0x5F3759DF        # Newton-rsqrt seed constant

# engine assignment tables (tuned against TimelineSim). GPSIMD (Pool) cannot
# touch PSUM, so PSUM evacuations split across ACT/DVE and Pool gets the
# SBUF-only work (xn quantize, residual adds).
UT_COPY_ENG = ["act", "act", "act", "dve"]      # per n-pair
FINADD_ENG = ["pool", "pool", "pool", "dve"]    # per d-tile
XN_ENG = "pool"
BODY_ORDER = "v1"   # v1: ATT(a),PROD(a),ATT(b),PROD(b); v2: ATT,ATT,PROD,PROD

_cache: dict = {}


def _dedup_ldweights(nc):
    """Drop InstLdweights that reload the identical weights AP."""
    ndrop = 0
    for f in nc.m.functions:
        for blk in f.blocks:
            insts = list(blk.instructions)
            drop = []
            last_key = None
            for idx, inst in enumerate(insts):
                nm = type(inst).__name__
                if nm == "InstLdweights":
                    si = inst.sync_info
                    has_sync = si is not None and (
                        len(si.on_wait) > 0 or len(si.on_update) > 0
                    )
                    key = str(inst.ins[0])
                    if key == last_key and not has_sync:
                        drop.append(idx)
                    else:
                        last_key = key
                elif nm == "InstMatmult":
                    if inst.perf_mode is None:
                        last_key = None
            for idx in reversed(drop):
                del blk.instructions[idx]
            ndrop += len(drop)
    return ndrop


def _build(loop_iters: int = 0):
    nc = bacc.Bacc("TRN2", target_bir_lowering=False, num_devices=NCORES)

    x_d = nc.dram_tensor("x", [BPC, C, N], F32R, kind="ExternalInput")
    wm_d = nc.dram_tensor("wm", [C, C], F8, kind="ExternalInput")   # 16*(Wq^T Wk)
    wu_d = nc.dram_tensor("wu", [C, C], F8, kind="ExternalInput")   # 16*(Wp Wv)^T
    ind_d = nc.dram_tensor("ind16", [C, G], F32R, kind="ExternalInput")  # 1/16 iff c//16==g
    bind_d = nc.dram_tensor("bind", [G, C], F32R, kind="ExternalInput")  # 0/1 indicator.T
    ones_d = nc.dram_tensor("onesm", [128, 256], F8, kind="ExternalInput")  # all 16.0
    consts_d = nc.dram_tensor("consts", [128, 2 * CT], F32, kind="ExternalInput")
    out_d = nc.dram_tensor("out", [BPC, C, N], F32, kind="ExternalOutput")

    halves = [slice(0, 512), slice(512, 1024)]

    with tile.TileContext(nc) as tc:
        with (
            tc.tile_pool(name="wpool", bufs=1) as wp_,
            tc.tile_pool(name="state", bufs=1) as stp,
            tc.tile_pool(name="epool", bufs=2) as epool,
            tc.tile_pool(name="fpool", bufs=4) as fpool,
            tc.tile_pool(name="rpool", bufs=2) as rpool,
            tc.tile_pool(name="spool", bufs=2) as spool,
            tc.tile_pool(name="ps2", bufs=2, space="PSUM") as ps2,
            tc.tile_pool(name="psA", bufs=3, space="PSUM") as psA,
            tc.tile_pool(name="psT", bufs=1, space="PSUM") as psT,
        ):
            # ---- persistent constants / weights (batched single DMAs) ----
            wm_all = wp_.tile([128, CT, C], F8, tag="wm", name="wm")
            wu_all = wp_.tile([128, CT, C], F8, tag="wu", name="wu")
            ind_all = wp_.tile([128, CT, G], F32R, tag="ind", name="ind")
            bind_all = wp_.tile([G, CT, 128], F32R, tag="bind", name="bind")
            ones_sb = wp_.tile([128, 2, 128], F8, tag="ones", name="ones")
            consts_sb = wp_.tile([128, 2 * CT], F32, tag="consts", name="consts")
            magic_sb = wp_.tile([128, 1], I32, tag="magic", name="magic")
            nc.vector.memset(magic_sb, MAGIC)
            exp_sc = wp_.tile([128, 1], F32, tag="expsc", name="expsc")
            nc.vector.memset(exp_sc, EXP_SCALE)
            exp_bi = wp_.tile([128, 1], F32, tag="expbi", name="expbi")
            nc.vector.memset(exp_bi, EXP_BIAS)
            gnsc_sb = consts_sb[:, 0 * CT:1 * CT]
            gnbi_sb = consts_sb[:, 1 * CT:2 * CT]

            def part(dram2d):
                return dram2d.rearrange("(t p) f -> p t f", p=128)

            nc.sync.dma_start(out=ind_all, in_=part(ind_d[:, :]))
            nc.sync.dma_start(out=consts_sb, in_=consts_d[:, :])
            nc.sync.dma_start(
                out=bind_all, in_=bind_d.rearrange("g (t p) -> g t p", p=128)
            )
            nc.sync.dma_start(out=wm_all, in_=part(wm_d[:, :]))
            nc.sync.dma_start(out=wu_all, in_=part(wu_d[:, :]))
            nc.sync.dma_start(
                out=ones_sb, in_=ones_d.rearrange("p (s f) -> p s f", s=2)
            )

            # per-(image, generation) persistent state: PROD writes one
            # generation while ATT consumes the other; the For_i body is
            # unrolled 2x so the generations alternate with static buffers.
            def mkstate(s):
                return {
                    "x": stp.tile([128, CT, N], F32R, tag=f"x{s}", name=f"x{s}"),
                    "xn": stp.tile([128, CT, N], F8, tag=f"xn{s}", name=f"xn{s}"),
                    "m": stp.tile([128, CT, N], F8, tag=f"m{s}", name=f"m{s}"),
                    "uT": stp.tile([128, NT, C], F8, tag=f"u{s}", name=f"u{s}"),
                }
            gens = [[mkstate("a0"), mkstate("b0")],
                    [mkstate("a1"), mkstate("b1")]]
            for g in gens:
                for i, st in enumerate(g):
                    st["img"] = i
            states = gens[0]

            def eng(name):
                return {"act": nc.scalar, "dve": nc.vector, "pool": nc.gpsimd}[name]

            def emit_gn(st):
                """bn_stats -> group agg -> a/b coefficients for one image.
                Small chain; caller interleaves the two images."""
                x_all = st["x"]
                xf = x_all.bitcast(F32)
                bst = spool.tile([128, 2 * CT, 6], F32, tag="bst", name="bst")
                for t in range(CT):
                    for h in range(2):
                        nc.vector.bn_stats(
                            out=bst[:, 2 * t + h, :], in_=xf[:, t, halves[h]]
                        )
                mv = spool.tile([128, CT, 2], F32, tag="mv", name="mv")
                for t in range(CT):
                    nc.vector.bn_aggr(
                        out=mv[:, t, :], in_=bst[:, 2 * t:2 * t + 2, :]
                    )
                msq = spool.tile([128, CT], F32, tag="msq", name="msq")
                nc.vector.tensor_mul(msq[:, :], mv[:, :, 0], mv[:, :, 0])
                m12 = spool.tile([128, CT, 2], F32R, tag="m12", name="m12")
                nc.vector.tensor_copy(m12[:, :, 0], mv[:, :, 0])
                nc.vector.tensor_add(m12[:, :, 1], mv[:, :, 1], msq[:, :])
                tiny = psT.tile([128, 16], F32, tag="tiny", name="tiny")
                gps = tiny[0:G, 0:2]
                for t in range(CT):
                    nc.tensor.matmul(
                        gps[:, :], ind_all[:, t, :], m12[:, t, :],
                        start=(t == 0), stop=(t == CT - 1),
                    )
                meang = spool.tile([G, 1], F32, tag="meang", name="meang")
                nc.vector.tensor_copy(meang[:, :], gps[:, 0:1])
                msqg = spool.tile([G, 1], F32, tag="msqg", name="msqg")
                vpe = spool.tile([G, 1], F32, tag="vpe", name="vpe")
                nc.vector.tensor_mul(msqg[:, :], meang[:, :], meang[:, :])
                nc.vector.scalar_tensor_tensor(
                    out=vpe[:, :], in0=gps[:, 1:2], scalar=EPS, in1=msqg[:, :],
                    op0=ALU.add, op1=ALU.subtract,
                )
                sh_t = spool.tile([G, 1], I32, tag="sh", name="sh")
                nc.vector.tensor_scalar(
                    out=sh_t[:, :], in0=vpe.bitcast(I32)[:, :], scalar1=1,
                    scalar2=None, op0=ALU.logical_shift_right,
                )
                seed = spool.tile([G, 1], I32, tag="seed", name="seed")
                nc.vector.scalar_tensor_tensor(
                    out=seed[:, :], in0=magic_sb[:G, :], scalar=0, in1=sh_t[:, :],
                    op0=ALU.bypass, op1=ALU.subtract,
                )
                y = seed.bitcast(F32)
                for it in range(2):
                    t1 = spool.tile([G, 1], F32, tag=f"nr{it}", name=f"nr{it}")
                    nc.vector.tensor_mul(t1[:, :], y[:, :], y[:, :])
                    nc.vector.tensor_mul(t1[:, :], t1[:, :], vpe[:, :])
                    nc.vector.tensor_scalar(
                        out=t1[:, :], in0=t1[:, :], scalar1=-0.5, scalar2=1.5,
                        op0=ALU.mult, op1=ALU.add,
                    )
                    y2 = spool.tile([G, 1], F32, tag=f"y{it}", name=f"y{it}")
                    nc.vector.tensor_mul(y2[:, :], y[:, :], t1[:, :])
                    y = y2
                stats2 = spool.tile([G, 2], F32R, tag="st2", name="st2")
                nc.vector.tensor_copy(stats2[:, 0:1], y[:, :])
                nc.vector.tensor_copy(stats2[:, 1:2], meang[:, :])
                bc_ps = tiny[:, 8:8 + 2 * CT]
                for t in range(CT):
                    nc.tensor.matmul(
                        bc_ps[:, 2 * t:2 * t + 2], bind_all[:, t, :], stats2[:, :],
                        start=True, stop=True,
                    )
                bc_v = bc_ps.rearrange("p (t s) -> p t s", s=2)
                a_all = spool.tile([128, CT], F32, tag="aall", name="aall")
                b_all = spool.tile([128, CT], F32, tag="ball", name="ball")
                nc.vector.tensor_mul(a_all[:, :], gnsc_sb, bc_v[:, :, 0])
                nc.vector.scalar_tensor_tensor(
                    out=b_all[:, :], in0=bc_v[:, :, 1], scalar=-1.0, in1=a_all[:, :],
                    op0=ALU.mult, op1=ALU.mult,
                )
                nc.vector.tensor_add(b_all[:, :], b_all[:, :], gnbi_sb)
                return a_all, b_all

            def prod_load(st0, st1):
                """x loads for both images, emitted at body start so the SP
                queue issues them before the fin-gated out DMAs."""
                for st in (st0, st1):
                    x_all = st["x"]
                    for t in range(CT):
                        for h in range(2):
                            nc.sync.dma_start(
                                out=x_all[:, t, halves[h]],
                                in_=x_d[st["img"], 128 * t:128 * (t + 1),
                                        halves[h]],
                            )

            def prod_gn(st0, st1):
                sts = [st0, st1]
                abs_ = [emit_gn(st0), emit_gn(st1)]
                # tiles 0-1 on DVE (immediately after the stats chain in
                # DVE's queue, unblocking the m/u projections' first pass);
                # tiles 2-3 on Pool in parallel
                for t in range(CT):
                    for s, st in enumerate(sts):
                        a_all, b_all = abs_[s]
                        e = "dve" if t < 2 else XN_ENG
                        eng(e).tensor_scalar(
                            out=st["xn"][:, t, :],
                            in0=st["x"].bitcast(F32)[:, t, :],
                            scalar1=a_all[:, t:t + 1],
                            scalar2=b_all[:, t:t + 1],
                            op0=ALU.mult, op1=ALU.add,
                        )

            def prod_mu(st0, st1):
                sts = [st0, st1]
                for d in range(CT):
                    ds_ = slice(128 * d, 128 * (d + 1))
                    for st in sts:
                        ps = ps2.tile([128, N], F32, tag="ps2", name="ps2")
                        for u in range(CT // 2):
                            us = slice(2 * u, 2 * u + 2)
                            for h in range(2):
                                nc.tensor.matmul(
                                    ps[:, halves[h]], wm_all[:, us, ds_],
                                    st["xn"][:, us, halves[h]],
                                    start=(u == 0), stop=(u == CT // 2 - 1),
                                    perf_mode=DR,
                                )
                        if d >= CT - 2:
                            nc.vector.tensor_copy(st["m"][:, d, :], ps[:, :])
                        else:
                            nc.scalar.copy(out=st["m"][:, d, :], in_=ps[:, :])
                for np_ in range(NT // 2):
                    for st in sts:
                        ps = ps2.tile([128, N], F32, tag="ps2", name="ps2")
                        psv = ps.rearrange("p (k f) -> p k f", k=2)
                        for k in range(2):
                            n = 2 * np_ + k
                            ns = slice(128 * n, 128 * (n + 1))
                            for u in range(CT // 2):
                                us = slice(2 * u, 2 * u + 2)
                                nc.tensor.matmul(
                                    psv[:, k, :], st["xn"][:, us, ns],
                                    wu_all[:, us, :],
                                    start=(u == 0), stop=(u == CT // 2 - 1),
                                    perf_mode=DR,
                                )
                        e = UT_COPY_ENG[np_]
                        if e == "act":
                            nc.scalar.copy(
                                out=st["uT"][:, 2 * np_:2 * np_ + 2, :],
                                in_=psv[:, :, :],
                            )
                        else:
                            eng(e).tensor_copy(
                                st["uT"][:, 2 * np_:2 * np_ + 2, :], psv[:, :, :]
                            )

            att_ctx = {}

            def att_fwd(st0, st1):
                """S/exp/L/recip for both images, tile-interleaved."""
                sts = [st0, st1]
                exps, recips = [], []
                for st in sts:
                    exps.append(epool.tile([128, NT, N], F8, tag="e", name="e"))
                    recips.append(rpool.tile([128, N], F32, tag="rbc", name="rbc"))
                for j in range(NT):
                    js = slice(128 * j, 128 * (j + 1))
                    for s, st in enumerate(sts):
                        ps = ps2.tile([128, N], F32, tag="ps2", name="ps2")
                        for u in range(CT // 2):
                            us = slice(2 * u, 2 * u + 2)
                            for h in range(2):
                                nc.tensor.matmul(
                                    ps[:, halves[h]], st["xn"][:, us, js],
                                    st["m"][:, us, halves[h]],
                                    start=(u == 0), stop=(u == CT // 2 - 1),
                                    perf_mode=DR,
                                )
                        nc.scalar.activation(out=exps[s][:, j, :], in_=ps[:, :],
                                             func=AF.Exp, scale=exp_sc[:, 0:1],
                                             bias=exp_bi[:, 0:1])
                lps = []
                for s in range(2):
                    ps = ps2.tile([128, N], F32, tag="ps2", name="ps2")
                    for u in range(NT // 2):
                        us = slice(2 * u, 2 * u + 2)
                        for h in range(2):
                            nc.tensor.matmul(
                                ps[:, halves[h]], ones_sb[:, :, :],
                                exps[s][:, us, halves[h]],
                                start=(u == 0), stop=(u == NT // 2 - 1),
                                perf_mode=DR,
                            )
                    lps.append(ps)
                for s in range(2):
                    nc.vector.reciprocal_approx_fast(out=recips[s][:, :],
                                                     in_=lps[s][:, :])
                att_ctx["exps"], att_ctx["recips"] = exps, recips

            def att_bwd(st0, st1):
                """attn@U, normalize, residual, writeback for both images."""
                sts = [st0, st1]
                exps, recips = att_ctx["exps"], att_ctx["recips"]
                for d in range(CT):
                    ds_ = slice(128 * d, 128 * (d + 1))
                    fins = []
                    for s, st in enumerate(sts):
                        fin = fpool.tile([128, N], F32, tag="fin", name="fin")
                        fins.append(fin)
                        for h in range(2):
                            ps = psA.tile([128, 512], F32, tag="psA", name="psA")
                            for u in range(NT // 2):
                                us = slice(2 * u, 2 * u + 2)
                                nc.tensor.matmul(
                                    ps[:, :], st["uT"][:, us, ds_],
                                    exps[s][:, us, halves[h]],
                                    start=(u == 0), stop=(u == NT // 2 - 1),
                                    perf_mode=DR,
                                )
                            nc.vector.tensor_mul(fins[s][:, halves[h]], ps[:, :],
                                                 recips[s][:, halves[h]])
                    for s, st in enumerate(sts):
                        xf = st["x"].bitcast(F32)
                        eng(FINADD_ENG[d]).tensor_add(
                            fins[s][:, :], fins[s][:, :], xf[:, d, :])
                        nc.scalar.dma_start(
                            out=out_d[st["img"], 128 * d:128 * (d + 1), :],
                            in_=fins[s][:, :],
                        )

            if loop_iters:
                # software-pipelined + 2x unrolled: the prologue produces
                # generation 0; each unrolled half consumes one generation
                # while producing the other, so no tight write-after-read
                # cycles on the state buffers. Total invocations = loop_iters.
                UNROLL = 8   # invocations per For_i iteration (amortizes
                             # the all-engine barrier at the loop back edge)
                assert loop_iters % UNROLL == 0
                prod_load(*gens[0])
                prod_gn(*gens[0])
                prod_mu(*gens[0])
                with tc.For_i(0, loop_iters // UNROLL, 1,
                              hint_engines=(mybir.EngineType.PE,
                                            mybir.EngineType.Activation,
                                            mybir.EngineType.DVE,
                                            mybir.EngineType.Pool,
                                            mybir.EngineType.SP)):
                    for rep in range(UNROLL // 2):
                        for g in (0, 1):
                            att_fwd(*gens[g])
                            prod_load(*gens[1 - g])
                            prod_gn(*gens[1 - g])
                            att_bwd(*gens[g])
                            prod_mu(*gens[1 - g])
            else:
                prod_load(*gens[0])
                prod_gn(*gens[0])
                prod_mu(*gens[0])
                att_fwd(*gens[0])
                att_bwd(*gens[0])

    _dedup_ldweights(nc)
    nc.compile()
    return nc


def _to_f8(a):
    return np.ascontiguousarray(
        np.clip(a, -240.0, 240.0).astype(ml_dtypes.float8_e4m3)
    )


def _prep_inputs(x, gn_scale, gn_bias, qkv_w, qkv_b, proj_w, proj_b):
    f = np.float32
    x_r = np.asarray(x, dtype=f).reshape(B, C, N)
    qkv_w = np.asarray(qkv_w, dtype=f)
    qkv_b = np.asarray(qkv_b, dtype=f)
    proj_w = np.asarray(proj_w, dtype=f)
    proj_b = np.asarray(proj_b, dtype=f)
    if np.any(qkv_b[0:2 * C]):
        raise NotImplementedError(
            "fused-weights kernel assumes zero q/k biases (reference uses zeros)"
        )
    # v-bias and proj-bias fold into a constant per-channel offset added to x
    # (rows of attn sum to 1): out += Wp @ bv + bp.
    bv = qkv_b[2 * C:3 * C]
    cvec = proj_w @ bv + proj_b
    if np.any(cvec):
        x_r = x_r + cvec[None, :, None]

    def col(v):
        return np.asarray(v, f).reshape(CT, 128).T

    consts = np.concatenate([col(gn_scale), col(gn_bias)], axis=1)
    indicator = (np.arange(C)[:, None] // GS == np.arange(G)[None, :]).astype(f)
    M = qkv_w[0:C].T @ qkv_w[C:2 * C]   # s_ij = xn_i^T M xn_j
    Wpr = proj_w @ qkv_w[2 * C:3 * C]   # u = W' xn
    common = {
        "wm": _to_f8(M * WSC),          # stationary [c,o]: m = wm^T xn = M^T xn
        "wu": _to_f8(Wpr.T * WSC),      # stationary [c,o]: u = wu^T xn = W' xn
        "ind16": np.ascontiguousarray(indicator / GS),
        "bind": np.ascontiguousarray(indicator.T),
        "onesm": np.full((128, 256), 16.0, dtype=ml_dtypes.float8_e4m3),
        "consts": np.ascontiguousarray(consts),
    }
    in_maps = []
    for i in range(NCORES):
        m = dict(common)
        m["x"] = np.ascontiguousarray(x_r[BPC * i:BPC * (i + 1)])
        in_maps.append(m)
    return in_maps, True


def kernel(x, gn_scale, gn_bias, qkv_w, qkv_b, proj_w, proj_b, _trace=False):
    in_maps, _ = _prep_inputs(x, gn_scale, gn_bias, qkv_w, qkv_b,
                              proj_w, proj_b)
    if "nc" not in _cache:
        _cache["nc"] = _build()
    nc = _cache["nc"]
    res = run_bass_kernel_spmd(nc, in_maps, core_ids=list(range(NCORES)),
                               trace=_trace)
    _cache["last_result"] = res
    out = np.stack([r["out"] for r in res.results], axis=0)
    return out.reshape(B, C, H, W)
